# revision 26
# baseline (speedup 1.0000x reference)
"""Multi-head attention (B=4, S=2048, D=1024, H=16, depth=64) on 8 TRN2 cores.

Sharding: core (b, g) = b*2 + g handles batch b and head-group g (8 heads).
Each core computes its 8 heads' attention plus the partial output
projection (Wo rows for its heads). Host sums the two head-group partials
per batch and adds bo + bv@Wo (softmax rows sum to 1, so V's bias
contributes the constant vector bv@Wo to every output row).

All PE inputs are fp16 (the PE streams 2-byte operands at full 2.4 GHz vs
half rate for 4-byte); accumulation stays fp32 in PSUM, softmax
normalization in fp32. exp is shifted by C=4 (cancels in the
normalization) so probabilities stay inside fp16 normal range.

Per-core phases:
  A : xT = x^T via fp16 PE transposes            [1024, 2048] (8 tiles)
  B0: V' = (x@Wv | ones-col per head)            [128, 8*65] x 16 t-chunks
  B1: fused QK projection: lhsT = [Wq_h | Wk_h]  -> QT/KT pair tiles
  B2: per head-pair, per q-quarter (512), per t-chunk:
        scoresT[t,q] both heads -> one [128,1024] psum tile
        PT = exp(scoresT*scale + mask_bias - 4)  one ACT op, fp16 out
        zT' += V'_h-slices @ PT                  K=128; psum row 64 = denom
      normalize zT by broadcast(1/denom)
  C : out_partial = sum_pairs zTn_pair^T @ Wo_pair
"""
import numpy as np

B, S, D = 4, 2048, 1024
H, E = 16, 64            # total heads, depth
HG = 8                   # heads per core (group)
NP = HG // 2             # head pairs per core
G = 2                    # head groups
NC_USED = 8
SCALE = 1.0 / 8.0
NEG_BIG = -1000000000.0
CSHIFT = 6.0             # exp shift, cancels in normalization; keeps
                         # exp(score*scale - CSHIFT) inside fp16 range
                         # (max scaled score on these inputs is ~15.2)

NT = S // 128            # 16 t-chunks
ND = D // 128            # 8 d-chunks
NQ = 4                   # 512-wide q slices over full S
QW = 512                 # q window in the attention loop
NQQ = S // QW            # 4

_cache = {}


def _build():
    import concourse.bass as bass
    import concourse.mybir as mybir
    import concourse.tile as tile
    from concourse import bacc
    from concourse.masks import make_identity

    F32 = mybir.dt.float32
    F16 = mybir.dt.float16
    EXP = mybir.ActivationFunctionType.Exp
    nc = bacc.Bacc()

    x = nc.declare_dram_parameter("x", [S, D], F32, isOutput=False)
    # wqk: per head h, cols [h*128, (h+1)*128) = [Wq_h | Wk_h] (host packs)
    wqk = nc.declare_dram_parameter("wqk", [D, HG * 2 * E], F32, isOutput=False)
    wv = nc.declare_dram_parameter("wv", [D, HG * E], F32, isOutput=False)
    bq = nc.declare_dram_parameter("bq", [HG * E], F32, isOutput=False)
    bk = nc.declare_dram_parameter("bk", [HG * E], F32, isOutput=False)
    wo = nc.declare_dram_parameter("wo", [HG * E, D], F32, isOutput=False)
    mb = nc.declare_dram_parameter("mb", [S], F32, isOutput=False)
    out = nc.declare_dram_parameter("out", [S, D], F32, isOutput=True)

    W = HG * E   # 512
    W2 = 2 * W   # 1024

    with tile.TileContext(nc) as tc:
        # SBUF: two LIFO stacks. Right: prologue tensors with nested
        # lifetimes. Left: tensors living to the end.
        const = tc.alloc_tile_pool(name="const", bufs=1)
        ident = const.tile([128, 128], F16)
        make_identity(nc, ident)
        mb_sb = const.tile([128, NT], F32)
        nc.sync.dma_start(out=mb_sb[:], in_=mb[:].rearrange("(c p) -> p c", p=128))
        bq_sb = const.tile([64, HG], F32)
        nc.sync.dma_start(out=bq_sb[:], in_=bq[:].rearrange("(h e) -> e h", e=E))
        bk_sb = const.tile([64, HG], F32)
        nc.sync.dma_start(out=bk_sb[:], in_=bk[:].rearrange("(h e) -> e h", e=E))
        ones_f = const.tile([128, 1], F32)
        nc.vector.memset(ones_f[:], 1.0)
        ones_r = const.tile([128, 1], F16)
        nc.vector.tensor_copy(ones_r[:], ones_f[:])

        xT_pool = tc.alloc_tile_pool(name="xT", bufs=1, side="right")
        xT = [xT_pool.tile([128, S], F16, name=f"xT{d}", tag=f"xT{d}") for d in range(ND)]
        wqk_p = tc.alloc_tile_pool(name="wqkp", bufs=1, side="right")
        wv_p = tc.alloc_tile_pool(name="wvp", bufs=1, side="right")
        wqk_sb = wqk_p.tile([128, ND * W2], F16)
        wv_sb = wv_p.tile([128, ND * W], F16)
        for d in range(ND):
            nc.gpsimd.dma_start(out=wv_sb[:, d * W:(d + 1) * W], in_=wv[d * 128:(d + 1) * 128, :])
            nc.gpsimd.dma_start(out=wqk_sb[:, d * W2:(d + 1) * W2], in_=wqk[d * 128:(d + 1) * 128, :])

        vp_pool = tc.alloc_tile_pool(name="vpp", bufs=1)
        vp = [vp_pool.tile([128, HG * (E + 1)], F16, name=f"vp{c}", tag=f"vp{c}")
              for c in range(NT)]

        # PSUM: tag "sc" [128,1024] x2 (4 banks) — scores + phase C;
        #       tag "sm" [128,512] x1 (1 bank) — V'/QK accumulation;
        #       psT x2 (phase A only); psZ "z" [65,512] x3.
        ps = tc.alloc_tile_pool(name="ps", bufs=1, space="PSUM")
        psT = tc.alloc_tile_pool(name="psT", bufs=2, space="PSUM")
        xload = tc.alloc_tile_pool(name="xload", bufs=6, side="right")

        # ---- V' ones columns (V data filled during phase A) ----
        for c in range(NT):
            for h in range(HG):
                nc.vector.tensor_copy(vp[c][:, h * (E + 1) + E:(h + 1) * (E + 1)],
                                      ones_r[:])

        # ---- QK projection, per 512-col slice: lhsT = [Wq_h | Wk_h] ----
        qt_pool = tc.alloc_tile_pool(name="qtp", bufs=1)
        kt_pool = tc.alloc_tile_pool(name="ktp", bufs=1)
        qt = [qt_pool.tile([128, S], F16, name=f"qt{p}", tag=f"qt{p}") for p in range(NP)]
        kt = [kt_pool.tile([128, S], F16, name=f"kt{p}", tag=f"kt{p}") for p in range(NP)]

        def build_qk_slice(p, s):
            for hh in range(2):
                h, lo = 2 * p + hh, hh * 64
                pq = ps.tile([128, 512], F32, name="pq", tag="sm")
                for d in range(ND):
                    c0 = d * W2 + h * 2 * E
                    nc.tensor.matmul(pq[:], wqk_sb[:, c0:c0 + 2 * E],
                                     xT[d][:, s * 512:(s + 1) * 512],
                                     start=(d == 0), stop=(d == ND - 1))
                nc.vector.tensor_scalar_add(qt[p][lo:lo + E, s * 512:(s + 1) * 512],
                                            pq[0:E, :], bq_sb[:, h:h + 1])
                nc.vector.tensor_scalar_add(kt[p][lo:lo + E, s * 512:(s + 1) * 512],
                                            pq[E:128, :], bk_sb[:, h:h + 1])

        wo_p = tc.alloc_tile_pool(name="wop", bufs=1)
        wo_sb = [wo_p.tile([128, D], F16, name=f"wo{p}", tag=f"wo{p}") for p in range(NP)]
        for p in range(NP):
            nc.gpsimd.dma_start(out=wo_sb[p][:], in_=wo[p * 128:(p + 1) * 128, :])

        pt_pool = tc.alloc_tile_pool(name="ptp", bufs=4)
        ztn_pool = tc.alloc_tile_pool(name="ztnp", bufs=1)
        ztn = [ztn_pool.tile([128, S], F16, name=f"ztn{p}", tag=f"ztn{p}")
               for p in range(NP)]
        nrm_pool = tc.alloc_tile_pool(name="nrm", bufs=2)
        stage = tc.alloc_tile_pool(name="stage", bufs=2)

        # ---- Phase A: load x (fp16), transpose to xT, fused V' + build0.
        # The V' matmuls and pair-0 QK build fill the PE while the x DMA
        # streams, so B2 starts ACT-bound right after phase A. ----
        for c in range(NT):
            xrow = xload.tile([128, D], F16, name="xrow", tag="xrow")
            nc.gpsimd.dma_start(out=xrow[:], in_=x[c * 128:(c + 1) * 128, :])
            for d in range(ND):
                ptr = psT.tile([128, 128], F16, name="ptr", tag="tr")
                nc.tensor.transpose(ptr[:], xrow[:, d * 128:(d + 1) * 128], ident[:])
                # split the PSUM->SBUF copies between DVE and the (still
                # idle) ACT engine so neither paces the prologue
                if d % 2 == 0:
                    nc.vector.tensor_copy(xT[d][:, c * 128:(c + 1) * 128], ptr[:])
                else:
                    nc.scalar.copy(xT[d][:, c * 128:(c + 1) * 128], ptr[:])
            pv = ps.tile([128, W], F32, name="pv", tag="sm")
            for d in range(ND):
                nc.tensor.matmul(pv[:], xT[d][:, c * 128:(c + 1) * 128],
                                 wv_sb[:, d * W:(d + 1) * W],
                                 start=(d == 0), stop=(d == ND - 1))
            for h in range(HG):
                nc.vector.tensor_copy(vp[c][:, h * (E + 1):h * (E + 1) + E],
                                      pv[:, h * E:(h + 1) * E])
            if c % 4 == 3:
                build_qk_slice(0, c // 4)
        xload.release()
        psT.release()

        psZ = tc.alloc_tile_pool(name="psZ", bufs=3, space="PSUM")

        # ---- B2 (+fused V' build, QK builds, and output projection) ----
        # p outer: pair p+1's QK build is emitted right after (p, qq=0) so it
        # hides under ~4 ACT-bound qq blocks; V' is fused into the first
        # block; the output projection for quarter qq runs right after the
        # last pair finishes that quarter.
        for p in range(NP):
            for qq in range(NQQ):
                q0 = qq * QW
                zts = [psZ.tile([E + 1, QW], F32, name=f"zt{hh}", tag="z")
                       for hh in range(2)]
                for c in range(NT):
                    sc = ps.tile([128, 2 * QW], F32, name="sc", tag="sc", bufs=2)
                    # both heads' scoresT (row groups 0 / 64), one shared exp
                    for hh in range(2):
                        lo = hh * E
                        nc.tensor.matmul(sc[:, hh * QW:(hh + 1) * QW],
                                         kt[p][lo:lo + E, c * 128:(c + 1) * 128],
                                         qt[p][lo:lo + E, q0:q0 + QW],
                                         start=True, stop=True)
                    pt = pt_pool.tile([128, 2 * QW], F16, name="pt", tag="pt")
                    nc.scalar.activation(pt[:], sc[:], EXP,
                                         bias=mb_sb[:, c:c + 1], scale=SCALE)
                    for hh in range(2):
                        h = 2 * p + hh
                        nc.tensor.matmul(zts[hh][:, :],
                                         vp[c][:, h * (E + 1):(h + 1) * (E + 1)],
                                         pt[:, hh * QW:(hh + 1) * QW],
                                         start=(c == 0), stop=(c == NT - 1))
                for hh in range(2):
                    lo = hh * E
                    # copy z rows + denominator row out of PSUM immediately so
                    # the zt slot frees for the next block; the (slower)
                    # reciprocal chain then runs off the critical path. The
                    # custom-DVE recip also misreads non-zero base partitions,
                    # so the denominator goes to a partition-0 tile.
                    dn = nrm_pool.tile([1, QW], F32, name="dn", tag="dn")
                    nc.vector.tensor_copy(dn[:], zts[hh][E:E + 1, :])
                    zc = nrm_pool.tile([E, QW], F32, name="zc", tag="zc")
                    nc.vector.tensor_copy(zc[:], zts[hh][0:E, :])
                    rr = nrm_pool.tile([1, QW], F32, name="rr", tag="rr")
                    scr = nrm_pool.tile([1, QW], F32, name="scr", tag="scr")
                    rb = nrm_pool.tile([E, QW], F32, name="rb", tag="rb")
                    nc.vector.reciprocal_approx_accurate(rr[:], dn[:], scr[:])
                    nc.gpsimd.partition_broadcast(rb[:], rr[:])
                    nc.vector.tensor_mul(ztn[p][lo:lo + E, q0:q0 + QW],
                                         zc[:], rb[:])
                if p == NP - 1:
                    # output projection for this q-quarter (all pairs done)
                    for cc in range(qq * 4, qq * 4 + 4):
                        st = stage.tile([128, D], F32, name="st", tag="st")
                        for s in range(2):
                            po = ps.tile([128, 512], F32, name="po", tag="sm")
                            for pp in range(NP):
                                nc.tensor.matmul(po[:],
                                                 ztn[pp][:, cc * 128:(cc + 1) * 128],
                                                 wo_sb[pp][:, s * 512:(s + 1) * 512],
                                                 start=(pp == 0), stop=(pp == NP - 1))
                            nc.vector.tensor_copy(st[:, s * 512:(s + 1) * 512], po[:])
                        nc.sync.dma_start(out=out[cc * 128:(cc + 1) * 128, :], in_=st[:])
            if p + 1 < NP:
                for s in range(NQ):
                    build_qk_slice(p + 1, s)

        wv_p.release()
        wqk_p.release()
        xT_pool.release()

        # release left-side pools in LIFO order
        stage.release()
        nrm_pool.release()
        ztn_pool.release()
        pt_pool.release()
        wo_p.release()
        kt_pool.release()
        qt_pool.release()
        psZ.release()
        ps.release()
        vp_pool.release()
        const.release()

    nc.compile()
    return nc


def _get_nc():
    if "nc" not in _cache:
        _cache["nc"] = _build()
    return _cache["nc"]


def _prep_in_maps(x, attention_mask, Wq, bq, Wk, bk, Wv, Wo):
    x = np.ascontiguousarray(x, dtype=np.float32)
    Wo = np.ascontiguousarray(np.asarray(Wo, np.float32))
    in_maps = []
    for b in range(B):
        mb_b = ((1.0 - np.asarray(attention_mask[b, :, 0], np.float32)) * NEG_BIG
                - CSHIFT).astype(np.float32)
        for g in range(G):
            hs = slice(g * HG, (g + 1) * HG)
            wqk_g = np.concatenate([np.asarray(Wq[hs], np.float32),
                                    np.asarray(Wk[hs], np.float32)], axis=2)
            in_maps.append({
                "x": x[b],
                "wqk": np.ascontiguousarray(
                    wqk_g.transpose(1, 0, 2).reshape(D, HG * 2 * E)),
                "wv": np.ascontiguousarray(
                    np.asarray(Wv[hs], np.float32).transpose(1, 0, 2).reshape(D, HG * E)),
                "bq": np.ascontiguousarray(np.asarray(bq[hs], np.float32).reshape(-1)),
                "bk": np.ascontiguousarray(np.asarray(bk[hs], np.float32).reshape(-1)),
                "wo": np.ascontiguousarray(Wo[g * HG * E:(g + 1) * HG * E, :]),
                "mb": mb_b,
            })
    return in_maps


def kernel(x, attention_mask, Wq, bq, Wk, bk, Wv, bv, Wo, bo):
    from concourse.bass_utils import run_bass_kernel_spmd

    Wo = np.ascontiguousarray(np.asarray(Wo, np.float32))
    in_maps = _prep_in_maps(x, attention_mask, Wq, bq, Wk, bk, Wv, Wo)
    nc = _get_nc()
    res = run_bass_kernel_spmd(nc, in_maps, list(range(NC_USED)))
    # host unshard: sum the two head-group partials per batch; add bo + bv@Wo
    bias = (np.asarray(bo, np.float32)
            + np.asarray(bv, np.float32).reshape(-1) @ Wo).astype(np.float32)
    outs = []
    for b in range(B):
        outs.append(res.results[2 * b]["out"] + res.results[2 * b + 1]["out"] + bias)
    return np.stack(outs).astype(np.float32)


# revision 27
# speedup vs baseline: 1.0402x; 1.0402x over previous
"""Multi-head attention (B=4, S=2048, D=1024, H=16, depth=64) on 8 TRN2 cores.

Sharding: core (b, g) = b*2 + g handles batch b and head-group g (8 heads).
Each core computes its 8 heads' attention plus the partial output
projection (Wo rows for its heads). Host sums the two head-group partials
per batch and adds bo + bv@Wo (softmax rows sum to 1, so V's bias
contributes the constant vector bv@Wo to every output row).

All PE inputs are fp16 (the PE streams 2-byte operands at full 2.4 GHz vs
half rate for 4-byte); accumulation stays fp32 in PSUM, softmax
normalization in fp32. exp is shifted by C=4 (cancels in the
normalization) so probabilities stay inside fp16 normal range.

Per-core phases:
  A : xT = x^T via fp16 PE transposes            [1024, 2048] (8 tiles)
  B0: V' = (x@Wv | ones-col per head)            [128, 8*65] x 16 t-chunks
  B1: fused QK projection: lhsT = [Wq_h | Wk_h]  -> QT/KT pair tiles
  B2: per head-pair, per q-quarter (512), per t-chunk:
        scoresT[t,q] both heads -> one [128,1024] psum tile
        PT = exp(scoresT*scale + mask_bias - 4)  one ACT op, fp16 out
        zT' += V'_h-slices @ PT                  K=128; psum row 64 = denom
      normalize zT by broadcast(1/denom)
  C : out_partial = sum_pairs zTn_pair^T @ Wo_pair
"""
import numpy as np

B, S, D = 4, 2048, 1024
H, E = 16, 64            # total heads, depth
HG = 8                   # heads per core (group)
NP = HG // 2             # head pairs per core
G = 2                    # head groups
NC_USED = 8
SCALE = 1.0 / 8.0
NEG_BIG = -1000000000.0
CSHIFT = 6.0             # exp shift, cancels in normalization; keeps
                         # exp(score*scale - CSHIFT) inside fp16 range
                         # (max scaled score on these inputs is ~15.2)

NT = S // 128            # 16 t-chunks
ND = D // 128            # 8 d-chunks
NQ = 4                   # 512-wide q slices over full S
QW = 512                 # q window in the attention loop
NQQ = S // QW            # 4

_cache = {}


def _build():
    import concourse.bass as bass
    import concourse.mybir as mybir
    import concourse.tile as tile
    from concourse import bacc
    from concourse.masks import make_identity

    F32 = mybir.dt.float32
    F16 = mybir.dt.float16
    EXP = mybir.ActivationFunctionType.Exp
    nc = bacc.Bacc()

    x = nc.declare_dram_parameter("x", [S, D], F32, isOutput=False)
    # wqk: per head h, cols [h*128, (h+1)*128) = [Wq_h | Wk_h] (host packs)
    wqk = nc.declare_dram_parameter("wqk", [D, HG * 2 * E], F32, isOutput=False)
    wv = nc.declare_dram_parameter("wv", [D, HG * E], F32, isOutput=False)
    bq = nc.declare_dram_parameter("bq", [HG * E], F32, isOutput=False)
    bk = nc.declare_dram_parameter("bk", [HG * E], F32, isOutput=False)
    wo = nc.declare_dram_parameter("wo", [HG * E, D], F32, isOutput=False)
    mb = nc.declare_dram_parameter("mb", [S], F32, isOutput=False)
    out = nc.declare_dram_parameter("out", [S, D], F32, isOutput=True)

    W = HG * E   # 512
    W2 = 2 * W   # 1024

    with tile.TileContext(nc) as tc:
        # SBUF: two LIFO stacks. Right: prologue tensors with nested
        # lifetimes. Left: tensors living to the end.
        const = tc.alloc_tile_pool(name="const", bufs=1)
        ident = const.tile([128, 128], F16)
        make_identity(nc, ident)
        mb_sb = const.tile([128, NT], F32)
        nc.sync.dma_start(out=mb_sb[:], in_=mb[:].rearrange("(c p) -> p c", p=128))
        bq_sb = const.tile([64, HG], F32)
        nc.sync.dma_start(out=bq_sb[:], in_=bq[:].rearrange("(h e) -> e h", e=E))
        bk_sb = const.tile([64, HG], F32)
        nc.sync.dma_start(out=bk_sb[:], in_=bk[:].rearrange("(h e) -> e h", e=E))
        ones_f = const.tile([128, 1], F32)
        nc.vector.memset(ones_f[:], 1.0)
        ones_r = const.tile([128, 1], F16)
        nc.vector.tensor_copy(ones_r[:], ones_f[:])

        xT_pool = tc.alloc_tile_pool(name="xT", bufs=1, side="right")
        xT = [xT_pool.tile([128, S], F16, name=f"xT{d}", tag=f"xT{d}") for d in range(ND)]
        wqk_p = tc.alloc_tile_pool(name="wqkp", bufs=1, side="right")
        wv_p = tc.alloc_tile_pool(name="wvp", bufs=1, side="right")
        wqk_sb = wqk_p.tile([128, ND * W2], F16)
        wv_sb = wv_p.tile([128, ND * W], F16)
        for d in range(ND):
            nc.gpsimd.dma_start(out=wv_sb[:, d * W:(d + 1) * W], in_=wv[d * 128:(d + 1) * 128, :])
            nc.gpsimd.dma_start(out=wqk_sb[:, d * W2:(d + 1) * W2], in_=wqk[d * 128:(d + 1) * 128, :])

        vp_pool = tc.alloc_tile_pool(name="vpp", bufs=1)
        vp = [vp_pool.tile([128, HG * (E + 1)], F16, name=f"vp{c}", tag=f"vp{c}")
              for c in range(NT)]

        # PSUM: tag "sc" [128,1024] x2 (4 banks) — scores + phase C;
        #       tag "sm" [128,512] x1 (1 bank) — V'/QK accumulation;
        #       psT x2 (phase A only); psZ "z" [65,512] x3.
        ps = tc.alloc_tile_pool(name="ps", bufs=1, space="PSUM")
        psT = tc.alloc_tile_pool(name="psT", bufs=2, space="PSUM")
        xload = tc.alloc_tile_pool(name="xload", bufs=3, side="right")

        # ---- V' ones columns (V data filled during phase A) ----
        for c in range(NT):
            for h in range(HG):
                nc.vector.tensor_copy(vp[c][:, h * (E + 1) + E:(h + 1) * (E + 1)],
                                      ones_r[:])

        # ---- QK projection, per 512-col slice: lhsT = [Wq_h | Wk_h] ----
        qt_pool = tc.alloc_tile_pool(name="qtp", bufs=1)
        kt_pool = tc.alloc_tile_pool(name="ktp", bufs=1)
        qt = [qt_pool.tile([128, S], F16, name=f"qt{p}", tag=f"qt{p}") for p in range(NP)]
        kt = [kt_pool.tile([128, S], F16, name=f"kt{p}", tag=f"kt{p}") for p in range(NP)]

        def build_qk_slice(p, s):
            for hh in range(2):
                h, lo = 2 * p + hh, hh * 64
                pq = ps.tile([128, 512], F32, name="pq", tag="sm")
                for d in range(ND):
                    c0 = d * W2 + h * 2 * E
                    nc.tensor.matmul(pq[:], wqk_sb[:, c0:c0 + 2 * E],
                                     xT[d][:, s * 512:(s + 1) * 512],
                                     start=(d == 0), stop=(d == ND - 1))
                nc.vector.tensor_scalar_add(qt[p][lo:lo + E, s * 512:(s + 1) * 512],
                                            pq[0:E, :], bq_sb[:, h:h + 1])
                nc.vector.tensor_scalar_add(kt[p][lo:lo + E, s * 512:(s + 1) * 512],
                                            pq[E:128, :], bk_sb[:, h:h + 1])

        wo_p = tc.alloc_tile_pool(name="wop", bufs=1)
        wo_sb = [wo_p.tile([128, D], F16, name=f"wo{p}", tag=f"wo{p}") for p in range(NP)]
        for p in range(NP):
            nc.gpsimd.dma_start(out=wo_sb[p][:], in_=wo[p * 128:(p + 1) * 128, :])

        pt_pool = tc.alloc_tile_pool(name="ptp", bufs=4)
        ztn_pool = tc.alloc_tile_pool(name="ztnp", bufs=1)
        ztn = [ztn_pool.tile([128, S], F16, name=f"ztn{p}", tag=f"ztn{p}")
               for p in range(NP)]
        nrm_pool = tc.alloc_tile_pool(name="nrm", bufs=2)
        stage = tc.alloc_tile_pool(name="stage", bufs=2)

        # ---- Phase A: load x (fp16), transpose to xT, fused V' + build0.
        # The V' matmuls and pair-0 QK build fill the PE while the x DMA
        # streams, so B2 starts ACT-bound right after phase A. ----
        for c in range(NT):
            xrow = xload.tile([128, D], F16, name="xrow", tag="xrow")
            nc.gpsimd.dma_start(out=xrow[:], in_=x[c * 128:(c + 1) * 128, :])
            for d in range(ND):
                ptr = psT.tile([128, 128], F16, name="ptr", tag="tr")
                nc.tensor.transpose(ptr[:], xrow[:, d * 128:(d + 1) * 128], ident[:])
                nc.vector.tensor_copy(xT[d][:, c * 128:(c + 1) * 128], ptr[:])
            pv = ps.tile([128, W], F32, name="pv", tag="sm")
            for d in range(ND):
                nc.tensor.matmul(pv[:], xT[d][:, c * 128:(c + 1) * 128],
                                 wv_sb[:, d * W:(d + 1) * W],
                                 start=(d == 0), stop=(d == ND - 1))
            for h in range(HG):
                nc.vector.tensor_copy(vp[c][:, h * (E + 1):h * (E + 1) + E],
                                      pv[:, h * E:(h + 1) * E])
            if c % 4 == 3:
                build_qk_slice(0, c // 4)
        xload.release()
        psT.release()

        psZ = tc.alloc_tile_pool(name="psZ", bufs=3, space="PSUM")

        # ---- B2 (+fused V' build, QK builds, and output projection) ----
        # p outer: pair p+1's QK build is emitted right after (p, qq=0) so it
        # hides under ~4 ACT-bound qq blocks; V' is fused into the first
        # block; the output projection for quarter qq runs right after the
        # last pair finishes that quarter.
        for p in range(NP):
            for qq in range(NQQ):
                q0 = qq * QW
                zts = [psZ.tile([E + 1, QW], F32, name=f"zt{hh}", tag="z")
                       for hh in range(2)]
                for c in range(NT):
                    sc = ps.tile([128, 2 * QW], F32, name="sc", tag="sc", bufs=2)
                    # both heads' scoresT (row groups 0 / 64), one shared exp
                    for hh in range(2):
                        lo = hh * E
                        nc.tensor.matmul(sc[:, hh * QW:(hh + 1) * QW],
                                         kt[p][lo:lo + E, c * 128:(c + 1) * 128],
                                         qt[p][lo:lo + E, q0:q0 + QW],
                                         start=True, stop=True)
                    pt = pt_pool.tile([128, 2 * QW], F16, name="pt", tag="pt")
                    nc.scalar.activation(pt[:], sc[:], EXP,
                                         bias=mb_sb[:, c:c + 1], scale=SCALE)
                    for hh in range(2):
                        h = 2 * p + hh
                        nc.tensor.matmul(zts[hh][:, :],
                                         vp[c][:, h * (E + 1):(h + 1) * (E + 1)],
                                         pt[:, hh * QW:(hh + 1) * QW],
                                         start=(c == 0), stop=(c == NT - 1))
                for hh in range(2):
                    lo = hh * E
                    # copy z rows + denominator row out of PSUM immediately so
                    # the zt slot frees for the next block; the (slower)
                    # reciprocal chain then runs off the critical path. The
                    # custom-DVE recip also misreads non-zero base partitions,
                    # so the denominator goes to a partition-0 tile.
                    dn = nrm_pool.tile([1, QW], F32, name="dn", tag="dn")
                    nc.vector.tensor_copy(dn[:], zts[hh][E:E + 1, :])
                    zc = nrm_pool.tile([E, QW], F32, name="zc", tag="zc")
                    nc.vector.tensor_copy(zc[:], zts[hh][0:E, :])
                    rr = nrm_pool.tile([1, QW], F32, name="rr", tag="rr")
                    scr = nrm_pool.tile([1, QW], F32, name="scr", tag="scr")
                    rb = nrm_pool.tile([E, QW], F32, name="rb", tag="rb")
                    nc.vector.reciprocal_approx_accurate(rr[:], dn[:], scr[:])
                    nc.gpsimd.partition_broadcast(rb[:], rr[:])
                    nc.vector.tensor_mul(ztn[p][lo:lo + E, q0:q0 + QW],
                                         zc[:], rb[:])
                if p == NP - 1:
                    # output projection for this q-quarter (all pairs done)
                    for cc in range(qq * 4, qq * 4 + 4):
                        st = stage.tile([128, D], F32, name="st", tag="st")
                        for s in range(2):
                            po = ps.tile([128, 512], F32, name="po", tag="sm")
                            for pp in range(NP):
                                nc.tensor.matmul(po[:],
                                                 ztn[pp][:, cc * 128:(cc + 1) * 128],
                                                 wo_sb[pp][:, s * 512:(s + 1) * 512],
                                                 start=(pp == 0), stop=(pp == NP - 1))
                            nc.vector.tensor_copy(st[:, s * 512:(s + 1) * 512], po[:])
                        nc.sync.dma_start(out=out[cc * 128:(cc + 1) * 128, :], in_=st[:])
            if p + 1 < NP:
                for s in range(NQ):
                    build_qk_slice(p + 1, s)

        wv_p.release()
        wqk_p.release()
        xT_pool.release()

        # release left-side pools in LIFO order
        stage.release()
        nrm_pool.release()
        ztn_pool.release()
        pt_pool.release()
        wo_p.release()
        kt_pool.release()
        qt_pool.release()
        psZ.release()
        ps.release()
        vp_pool.release()
        const.release()

    nc.compile()
    return nc


def _get_nc():
    if "nc" not in _cache:
        _cache["nc"] = _build()
    return _cache["nc"]


def _prep_in_maps(x, attention_mask, Wq, bq, Wk, bk, Wv, Wo):
    x = np.ascontiguousarray(x, dtype=np.float32)
    Wo = np.ascontiguousarray(np.asarray(Wo, np.float32))
    in_maps = []
    for b in range(B):
        mb_b = ((1.0 - np.asarray(attention_mask[b, :, 0], np.float32)) * NEG_BIG
                - CSHIFT).astype(np.float32)
        for g in range(G):
            hs = slice(g * HG, (g + 1) * HG)
            wqk_g = np.concatenate([np.asarray(Wq[hs], np.float32),
                                    np.asarray(Wk[hs], np.float32)], axis=2)
            in_maps.append({
                "x": x[b],
                "wqk": np.ascontiguousarray(
                    wqk_g.transpose(1, 0, 2).reshape(D, HG * 2 * E)),
                "wv": np.ascontiguousarray(
                    np.asarray(Wv[hs], np.float32).transpose(1, 0, 2).reshape(D, HG * E)),
                "bq": np.ascontiguousarray(np.asarray(bq[hs], np.float32).reshape(-1)),
                "bk": np.ascontiguousarray(np.asarray(bk[hs], np.float32).reshape(-1)),
                "wo": np.ascontiguousarray(Wo[g * HG * E:(g + 1) * HG * E, :]),
                "mb": mb_b,
            })
    return in_maps


def kernel(x, attention_mask, Wq, bq, Wk, bk, Wv, bv, Wo, bo):
    from concourse.bass_utils import run_bass_kernel_spmd

    Wo = np.ascontiguousarray(np.asarray(Wo, np.float32))
    in_maps = _prep_in_maps(x, attention_mask, Wq, bq, Wk, bk, Wv, Wo)
    nc = _get_nc()
    res = run_bass_kernel_spmd(nc, in_maps, list(range(NC_USED)))
    # host unshard: sum the two head-group partials per batch; add bo + bv@Wo
    bias = (np.asarray(bo, np.float32)
            + np.asarray(bv, np.float32).reshape(-1) @ Wo).astype(np.float32)
    outs = []
    for b in range(B):
        outs.append(res.results[2 * b]["out"] + res.results[2 * b + 1]["out"] + bias)
    return np.stack(outs).astype(np.float32)
